# revision 1
# baseline (speedup 1.0000x reference)
"""GCN SpMM kernel for Trainium2 (8 NeuronCores, SPMD).

Computation (see reference):
    h = x @ W.T
    deg[n] = #edges with col==n;  dis = rsqrt(deg) (0 where deg==0)
    norm_e = dis[row_e] * dis[col_e]
    out[r] = sum_{e: row_e==r} norm_e * h[col_e]

Linearity lets us aggregate x first and project once per output row:
    out = (segment_sum(norm_e * x[col_e], row_e)) @ W.T

Sharding: output rows (and their edges) are split across 8 cores; x is
replicated into every core's DRAM at input-staging time (the "halo
all-gather" is satisfied by replication since sources are random).

Per-core device program:
  - edges sorted by local dest, grouped into windows of 128 dest rows,
    padded per-window to a multiple of 128 edges (pad: src=0, norm=0)
  - batched indirect DMA gather: G[p, c*128:(c+1)*128] = x[src[p, c]]
  - per 128-edge chunk, one fused DVE op builds the scatter matrix
        S[e, d] = (iota[d] == dest_e) * norm_e
  - PE matmul accumulates aggT[f, d] += G_c[e, f]^T @ S_c[e, d] in PSUM
    over a window's chunks, then projects out2[of, d] = W^T-stationary
    matmul; results collect in an SBUF tile, one DMA writes them out.
"""

import sys

sys.path.insert(0, "/opt/trn_rl_repo")

import numpy as np

import concourse.bass as bass
import concourse.mybir as mybir
import concourse.tile as tile
from concourse.alu_op_type import AluOpType
from concourse.bass import IndirectOffsetOnAxis
from concourse.bass_utils import run_bass_kernel_spmd
from concourse.vector_clock import ScopedClock

# ---------------------------------------------------------------------------
# Workaround: this walrus build rejects instructions with >1 sync wait
# ("Too many sync wait commands"). TileContext's kernel-tail drain can carry
# several; split them across multiple drain instructions.
_MAX_WAITS = 1


def _split_drain_and_barrier(self, tick_clock, wait_clock):
    nc = self.nc
    drain_inst = nc.sync.drain()
    wait_clock.add_sem_waits(
        drain_inst.ins, ScopedClock({None: tick_clock.global_clock})
    )
    si = drain_inst.ins.sync_info
    if si is not None and si.on_wait and len(si.on_wait) > _MAX_WAITS:
        waits = list(si.on_wait)
        si.on_wait = waits[:_MAX_WAITS]
        rest = waits[_MAX_WAITS:]
        while rest:
            d2 = nc.sync.drain()
            si2 = d2.ins.sync_info
            if si2 is None:
                d2.ins.sync_info = mybir.SyncInfo(
                    on_wait=rest[:_MAX_WAITS], on_update=[]
                )
            else:
                si2.on_wait = rest[:_MAX_WAITS]
            rest = rest[_MAX_WAITS:]
    nc.all_engine_barrier()
    assert self.sems is not None
    popped = nc._tile_sem_poison_stack.pop()
    assert popped is self._sem_poison
    nc.clear_and_free_semaphores(list(self.sems.allocated().values()))
    nc.all_engine_barrier()


tile.TileContext._drain_and_barrier = _split_drain_and_barrier

# Enable walrus DGE dynamic DMA so indirect (vector-offset) DMAs lower to
# real descriptor-generation DMAs instead of per-index unrolled Pool code.
import concourse.bass_utils as _bass_utils

_orig_get_walrus_args = _bass_utils.get_walrus_args


def _patched_walrus_args(*args, **kwargs):
    return [
        "--dge-levels=io,spill_reload,scalar_dynamic_offset,vector_dynamic_offsets",
        *_orig_get_walrus_args(*args, **kwargs),
    ]


_bass_utils.get_walrus_args = _patched_walrus_args


def split_multi_waits(nc):
    """Split any instruction carrying >1 sem waits: move extra waits onto
    same-engine NOPs inserted immediately before it (engines execute their
    instructions in block order, so this is equivalent)."""
    n = 0
    for bb in nc.m.functions[0].blocks:
        new_insts = []
        for ins in bb.instructions:
            si = ins.sync_info
            if si is not None and si.on_wait and len(si.on_wait) > 1:
                waits = list(si.on_wait)
                for w in waits[:-1]:
                    n += 1
                    nop = mybir.InstNoOp(
                        name=f"waitsplit-{n}-{ins.name}",
                        sync_info=mybir.SyncInfo(on_wait=[w], on_update=[]),
                        bass_nofuse=True,
                        engine=ins.engine,
                    )
                    new_insts.append(nop)
                si.on_wait = waits[-1:]
            new_insts.append(ins)
        bb.instructions[:] = new_insts
    return n


# ---------------------------------------------------------------------------

N_NODES = 100000
F = 128
N_CORES = 8
ROWS_PER_CORE = N_NODES // N_CORES  # 12500
WIN = 128  # dest rows per window
P = 128  # partitions / edges per chunk
CB = 32  # max chunks per gather batch (4096 descs: sem inc fits 16-bit field)


def host_prep(x, W, edge_index):
    """Compute norms, sort/pad edges per core, build device-layout arrays."""
    row = np.asarray(edge_index[0], dtype=np.int64)
    col = np.asarray(edge_index[1], dtype=np.int64)
    x = np.ascontiguousarray(np.asarray(x, dtype=np.float32))
    W = np.asarray(W, dtype=np.float32)

    deg = np.bincount(col, minlength=N_NODES).astype(np.float64)
    with np.errstate(divide="ignore"):
        dis = np.where(deg > 0, 1.0 / np.sqrt(deg), 0.0).astype(np.float32)
    norm = dis[row] * dis[col]

    n_win = (ROWS_PER_CORE + WIN - 1) // WIN  # 98

    per_core = []
    counts = np.zeros((N_CORES, n_win), dtype=np.int64)
    core_of = row // ROWS_PER_CORE
    for k in range(N_CORES):
        m = core_of == k
        r = (row[m] - k * ROWS_PER_CORE).astype(np.int64)
        c = col[m].astype(np.int32)
        nm = norm[m]
        order = np.argsort(r, kind="stable")
        r, c, nm = r[order], c[order], nm[order]
        w = r >> 7
        counts[k] = np.bincount(w, minlength=n_win)
        per_core.append((r, c, nm))

    cw = (counts.max(axis=0) + P - 1) // P  # chunks per window, shared
    cw = np.maximum(cw, 1).astype(np.int64)
    c0 = np.concatenate([[0], np.cumsum(cw)[:-1]])  # first chunk of window
    c_total = int(cw.sum())

    idx_all = np.zeros((N_CORES, c_total, P), dtype=np.int32)
    dst_all = np.zeros((N_CORES, c_total, P), dtype=np.float32)
    nrm_all = np.zeros((N_CORES, c_total, P), dtype=np.float32)
    for k in range(N_CORES):
        r, c, nm = per_core[k]
        w = r >> 7
        win_start = np.concatenate([[0], np.cumsum(counts[k])[:-1]])
        j = np.arange(len(r)) - win_start[w]
        chunk = c0[w] + (j >> 7)
        part = j & (P - 1)
        idx_all[k, chunk, part] = c
        dst_all[k, chunk, part] = (r & (WIN - 1)).astype(np.float32)
        nrm_all[k, chunk, part] = nm

    # [core, 128, c_total] layout: partition-major for the SBUF index tiles
    idx_all = np.ascontiguousarray(idx_all.transpose(0, 2, 1))
    dst_all = np.ascontiguousarray(dst_all.transpose(0, 2, 1))
    nrm_all = np.ascontiguousarray(nrm_all.transpose(0, 2, 1))

    wt = np.ascontiguousarray(W.T)  # lhsT[f, of] = W[of, f]
    iota = np.tile(np.arange(WIN, dtype=np.float32), (P, 1))
    return x, wt, iota, idx_all, dst_all, nrm_all, cw, c0, c_total, n_win


def build_program(c_total, cw, n_win, split_waits=True, mode="full"):
    """Build the per-core Bass/Tile program (same for all cores)."""
    nc = bass.Bass("TRN2", target_bir_lowering=False, debug=False, num_devices=1)
    dt = mybir.dt

    x_d = nc.dram_tensor("x", [N_NODES, F], dt.float32, kind="ExternalInput")
    wt_d = nc.dram_tensor("wt", [F, F], dt.float32, kind="ExternalInput")
    iota_d = nc.dram_tensor("iota", [P, WIN], dt.float32, kind="ExternalInput")
    idx_d = nc.dram_tensor("idx", [P, c_total], dt.int32, kind="ExternalInput")
    dst_d = nc.dram_tensor("dst", [P, c_total], dt.float32, kind="ExternalInput")
    nrm_d = nc.dram_tensor("nrm", [P, c_total], dt.float32, kind="ExternalInput")
    y_d = nc.dram_tensor("y", [F, n_win * WIN], dt.float32, kind="ExternalOutput")

    # batches: contiguous runs of <= CB chunks
    batches = []
    s = 0
    while s < c_total:
        b = min(CB, c_total - s)
        batches.append((s, b))
        s += b
    batch_of_chunk = np.zeros(c_total, dtype=np.int64)
    for bi, (s, b) in enumerate(batches):
        batch_of_chunk[s : s + b] = bi

    with tile.TileContext(nc) as tc:
        with (
            tc.tile_pool(name="const", bufs=1) as const_pool,
            tc.tile_pool(name="out", bufs=1) as out_pool,
            tc.tile_pool(name="gather", bufs=16) as g_pool,
            tc.tile_pool(name="meta", bufs=2) as meta_pool,
            tc.tile_pool(name="s", bufs=4) as s_pool,
            tc.tile_pool(name="aggsb", bufs=2) as aggsb_pool,
            tc.tile_pool(name="psum_agg", bufs=2, space="PSUM") as pa_pool,
            tc.tile_pool(name="psum_proj", bufs=2, space="PSUM") as pp_pool,
        ):
            wt_sb = const_pool.tile([F, F], dt.float32, tag="wt")
            nc.sync.dma_start(out=wt_sb[:], in_=wt_d.ap())
            iota_sb = const_pool.tile([P, WIN], dt.float32, tag="iota")
            nc.sync.dma_start(out=iota_sb[:], in_=iota_d.ap())
            out_sb = out_pool.tile([F, n_win * WIN], dt.float32, tag="out")

            idx_tiles = [None] * len(batches)
            dst_tiles = [None] * len(batches)
            nrm_tiles = [None] * len(batches)
            batch_start = [s for s, _ in batches]

            def emit_batch(bi):
                s, b = batches[bi]
                idx_t = meta_pool.tile([P, CB], dt.int32, tag="idx")
                nc.sync.dma_start(out=idx_t[:, :b], in_=idx_d.ap()[:, s : s + b])
                dst_t = meta_pool.tile([P, CB], dt.float32, tag="dst")
                nc.sync.dma_start(out=dst_t[:, :b], in_=dst_d.ap()[:, s : s + b])
                nrm_t = meta_pool.tile([P, CB], dt.float32, tag="nrm")
                nc.sync.dma_start(out=nrm_t[:, :b], in_=nrm_d.ap()[:, s : s + b])
                idx_tiles[bi] = idx_t
                dst_tiles[bi] = dst_t
                nrm_tiles[bi] = nrm_t

            emit_batch(0)
            g_fixed = None
            if mode == "nogather":
                g_fixed = g_pool.tile([P, F], dt.float32, tag="gf")
                nc.sync.dma_start(out=g_fixed[:], in_=x_d.ap()[0:P, :])
            c0 = np.concatenate([[0], np.cumsum(cw)[:-1]])
            for w in range(n_win):
                if mode == "gatheronly":
                    for i in range(int(cw[w])):
                        c = int(c0[w]) + i
                        bi = int(batch_of_chunk[c])
                        if idx_tiles[bi] is None:
                            emit_batch(bi)
                        cb = c - batch_start[bi]
                        g_c = g_pool.tile([P, F], dt.float32, tag="g")
                        nc.gpsimd.indirect_dma_start(
                            out=g_c[:],
                            out_offset=None,
                            in_=x_d.ap(),
                            in_offset=IndirectOffsetOnAxis(
                                ap=idx_tiles[bi][:, cb : cb + 1], axis=0
                            ),
                        )
                    continue
                agg = pa_pool.tile([F, WIN], dt.float32, tag="agg")
                n_c = int(cw[w])
                for i in range(n_c):
                    c = int(c0[w]) + i
                    bi = int(batch_of_chunk[c])
                    if idx_tiles[bi] is None:
                        emit_batch(bi)
                    cb = c - batch_start[bi]
                    # per-chunk gather: one offset per partition (the only
                    # vector-offset shape this walrus DGE lowers correctly)
                    if mode == "nogather":
                        g_c = g_fixed
                    else:
                        g_c = g_pool.tile([P, F], dt.float32, tag="g")
                        nc.gpsimd.indirect_dma_start(
                            out=g_c[:],
                            out_offset=None,
                            in_=x_d.ap(),
                            in_offset=IndirectOffsetOnAxis(
                                ap=idx_tiles[bi][:, cb : cb + 1], axis=0
                            ),
                        )
                    if mode == "gatheronly":
                        continue
                    s_t = s_pool.tile([P, WIN], dt.float32, tag="s")
                    nc.vector.scalar_tensor_tensor(
                        out=s_t[:],
                        in0=iota_sb[:],
                        scalar=dst_tiles[bi][:, cb : cb + 1],
                        in1=nrm_tiles[bi][:, cb : cb + 1].to_broadcast([P, WIN]),
                        op0=AluOpType.is_equal,
                        op1=AluOpType.mult,
                    )
                    nc.tensor.matmul(
                        agg[:],
                        lhsT=g_c[:],
                        rhs=s_t[:],
                        start=(i == 0),
                        stop=(i == n_c - 1),
                    )
                agg_sb = aggsb_pool.tile([F, WIN], dt.float32, tag="aggsb")
                nc.vector.tensor_copy(out=agg_sb[:], in_=agg[:])
                proj = pp_pool.tile([F, WIN], dt.float32, tag="proj")
                nc.tensor.matmul(
                    proj[:], lhsT=wt_sb[:], rhs=agg_sb[:], start=True, stop=True
                )
                nc.vector.tensor_copy(
                    out=out_sb[:, w * WIN : (w + 1) * WIN], in_=proj[:]
                )
            if mode != "gatheronly":
                nc.sync.dma_start(out=y_d.ap(), in_=out_sb[:])
    if split_waits:
        split_multi_waits(nc)
    return nc


def kernel(x, W, edge_index):
    x, wt, iota, idx_all, dst_all, nrm_all, cw, c0, c_total, n_win = host_prep(
        x, W, edge_index
    )
    nc = build_program(c_total, cw, n_win)
    in_maps = [
        {
            "x": x,
            "wt": wt,
            "iota": iota,
            "idx": idx_all[k],
            "dst": dst_all[k],
            "nrm": nrm_all[k],
        }
        for k in range(N_CORES)
    ]
    res = run_bass_kernel_spmd(nc, in_maps, core_ids=list(range(N_CORES)))
    outs = []
    for k in range(N_CORES):
        y_t = res.results[k]["y"]  # [F, n_win*WIN] feat-major
        outs.append(np.ascontiguousarray(y_t[:, :ROWS_PER_CORE].T))
    return np.concatenate(outs, axis=0)

